# revision 1
# baseline (speedup 1.0000x reference)
"""DIEN (GRU + AUGRU scan) Trainium2 Bass kernel.

Strategy
--------
Data-parallel over batch: B=256 is split 8 ways (32 per core). All weights are
replicated. The sequential scan over T=200 runs locally per core.

Algebraic fusion (host-side, exact):
  The per-step attention is over a length-1 sequence, so softmax==1 and
  attn == v_proj(g). Folding v_proj into the AUGRU input weights:
    aug_in @ augru_Wih.T == g @ (A1 + A2 @ v_W).T + (augru_bih + A2 @ v_b)
  with A1 = augru_Wih[:, :H], A2 = augru_Wih[:, H:]. This removes the v-proj
  matmul and halves the AUGRU input GEMM.

Per-step compute per core: 4 GEMM groups of [32,512] @ [512,1536]
(x-projection, GRU-hidden, fused AUGRU-input, AUGRU-hidden). Each group is
mapped PE-efficiently with the batch (32) as the stationary free dim using
4x column tiling (tile_position=(0,32c)): 4 concurrent matmuls per K-tile,
each pumping 384 weight columns.

Layouts (per core, batch b in 0..31, hidden h = 128*c + 32*m + jr):
  row layout  : tile[32*c + b, 32*m + jr]  (states, gates, psum outputs)
  stationary  : tileT[32*c + jr, 32*m + b] -- obtained from row layout by a
                single DVE 32x32 block transpose; K-tile m of the GEMM
                contracts hidden dims {128c + 32m + jr}, and the weight
                matrices are pre-arranged (host-side numpy) to match.
"""

import os
import sys

import numpy as np

for _p in ("/opt/trn_rl_repo", "/root/.axon_site/_ro/trn_rl_repo"):
    if os.path.isdir(_p) and _p not in sys.path:
        sys.path.append(_p)

B, T, H = 256, 200, 512
N_CORES = 8
BL = B // N_CORES  # 32

_CACHE = {}


# ---------------------------------------------------------------------------
# Host-side weight preparation (pure numpy, exact rearrangements)
# ---------------------------------------------------------------------------

def _arrange_w(W):
    """[3H, H] (out, in) -> [4, 128, 1536] K-tile-arranged weight blocks.

    Block m, partition p = 32*c_in + jr holds input dim h_in = 128*c_in + 32*m + jr.
    Free index f = c_out*384 + gate*128 + j  maps output col gate*512 + c_out*128 + j.
    """
    A = W.T.reshape(4, 4, 32, 3 * H)                # [c_in, m, jr, out]
    A = A.transpose(1, 0, 2, 3).reshape(4, 128, 3 * H)
    A = A.reshape(4, 128, 3, 4, 128).transpose(0, 1, 3, 2, 4).reshape(4, 128, 3 * H)
    A = A.transpose(1, 0, 2)                        # [p, m, out] for contiguous DMA
    return np.ascontiguousarray(A, dtype=np.float32)


def _bias_rz(bv):
    """[3H] -> [128, 256] broadcast tile for the r,z gate chunks."""
    v = bv[:1024].reshape(2, 4, 128).transpose(1, 0, 2).reshape(4, 256)
    return np.ascontiguousarray(np.repeat(v, 32, axis=0), dtype=np.float32)


def _bias_n(bv):
    """[3H] -> [128, 128] broadcast tile for the n gate chunk."""
    v = bv[1024:].reshape(4, 128)
    return np.ascontiguousarray(np.repeat(v, 32, axis=0), dtype=np.float32)


# ---------------------------------------------------------------------------
# Bass program
# ---------------------------------------------------------------------------

def _build_program(n_steps=T, hh_order="sweep", de_engine="pool", ah_early=False):
    import concourse.bacc as bacc
    import concourse.tile as tile
    from concourse import mybir
    from contextlib import ExitStack

    F32 = mybir.dt.float32
    Sigmoid = mybir.ActivationFunctionType.Sigmoid
    Tanh = mybir.ActivationFunctionType.Tanh

    nc = bacc.Bacc("TRN2", target_bir_lowering=False, debug=False)

    seq = nc.declare_dram_parameter("seq", [BL, n_steps, H], F32, isOutput=False)
    w_dram = {
        name: nc.declare_dram_parameter(name, [128, 4, 3 * H], F32, isOutput=False)
        for name in ("wgi", "wgh", "wai", "wah")
    }
    b_dram = {}
    for name, cols in (
        ("brz_g", 256), ("bihn_g", 128), ("bhhn_g", 128),
        ("brz_a", 256), ("bihn_a", 128), ("bhhn_a", 128),
    ):
        b_dram[name] = nc.declare_dram_parameter(name, [128, cols], F32, isOutput=False)
    out = nc.declare_dram_parameter("out", [BL, H], F32, isOutput=True)

    with tile.TileContext(nc) as tc, ExitStack() as ctx:
        wpool = ctx.enter_context(tc.tile_pool(name="weights", bufs=1))
        xrow_pool = ctx.enter_context(tc.tile_pool(name="xrow", bufs=6))
        xt_pool = ctx.enter_context(tc.tile_pool(name="xt", bufs=6))
        st_pool = ctx.enter_context(tc.tile_pool(name="states", bufs=3))
        tmp_pool = ctx.enter_context(tc.tile_pool(name="tmps", bufs=3))
        psum_pool = ctx.enter_context(tc.tile_pool(name="psum", bufs=2, space="PSUM"))

        # --- constants: weights + biases ---
        wsb = {}
        for name, drm in w_dram.items():
            t = wpool.tile([128, 4 * 3 * H], F32, tag=name)
            nc.sync.dma_start(out=t, in_=drm[:].rearrange("p m f -> p (m f)"))
            wsb[name] = t
        bsb = {}
        for name, drm in b_dram.items():
            t = wpool.tile([128, drm.shape[1]], F32, tag=name)
            nc.sync.dma_start(out=t, in_=drm[:])
            bsb[name] = t

        # --- initial states (zero) ---
        g_row = st_pool.tile([128, 128], F32, tag="g_row")
        gT = st_pool.tile([128, 128], F32, tag="gT")
        a_row = st_pool.tile([128, 128], F32, tag="a_row")
        aT = st_pool.tile([128, 128], F32, tag="aT")
        for t_ in (g_row, gT, a_row, aT):
            nc.vector.memset(t_, 0.0)

        def mm_group(psum, statT, w, starts=True):
            """psum[32c+b, :384] += statT-K-tiles.T @ w chunks (full r|z|n)."""
            for k in range(4):
                lhsT = statT[:, 32 * k:32 * k + 32]
                for c in range(4):
                    nc.tensor.matmul(
                        out=psum[32 * c:32 * c + 32, :],
                        lhsT=lhsT,
                        rhs=w[:, k * 1536 + 384 * c:k * 1536 + 384 * c + 384],
                        start=(starts and k == 0),
                        stop=(not starts and k == 3),
                        skip_group_check=True,
                        tile_position=(0, 32 * c),
                    )

        def mm_group_hh(psum_rz, psum_n, statT, w):
            """Hidden-side group: r,z part accumulates onto the input-side psum
            (saves a PSUM+PSUM add the DVE cannot do); n part goes to its own
            psum tile so r can gate it before the tanh.

            Emission order matters: the PE is strict FIFO and a matmul on col
            tile c blocks behind an unfinished matmul on the same tile, so we
            sweep c within each pump type to keep the 4 col tiles concurrent.
            """
            for k in range(4):
                lhsT = statT[:, 32 * k:32 * k + 32]
                if hh_order == "sweep":
                    for c in range(4):
                        base = k * 1536 + 384 * c
                        nc.tensor.matmul(
                            out=psum_rz[32 * c:32 * c + 32, 0:256],
                            lhsT=lhsT,
                            rhs=w[:, base:base + 256],
                            start=False, stop=(k == 3), skip_group_check=True,
                            tile_position=(0, 32 * c),
                        )
                    for c in range(4):
                        base = k * 1536 + 384 * c
                        nc.tensor.matmul(
                            out=psum_n[32 * c:32 * c + 32, :],
                            lhsT=lhsT,
                            rhs=w[:, base + 256:base + 384],
                            start=(k == 0), stop=(k == 3), skip_group_check=True,
                            tile_position=(0, 32 * c),
                        )
                else:
                    for c in range(4):
                        base = k * 1536 + 384 * c
                        nc.tensor.matmul(
                            out=psum_rz[32 * c:32 * c + 32, 0:256],
                            lhsT=lhsT,
                            rhs=w[:, base:base + 256],
                            start=False, stop=(k == 3), skip_group_check=True,
                            tile_position=(0, 32 * c),
                        )
                        nc.tensor.matmul(
                            out=psum_n[32 * c:32 * c + 32, :],
                            lhsT=lhsT,
                            rhs=w[:, base + 256:base + 384],
                            start=(k == 0), stop=(k == 3), skip_group_check=True,
                            tile_position=(0, 32 * c),
                        )

        def cell(pgi, pghn, row_prev, brz, bihn, bhhn, row_tag):
            # pgi[:, 0:256] already holds gi_rz + gh_rz (PE-accumulated)
            s2 = tmp_pool.tile([128, 256], F32, tag=row_tag + "s2")
            nc.vector.tensor_add(s2, pgi[:, 0:256], brz)
            rz = tmp_pool.tile([128, 256], F32, tag=row_tag + "rz")
            nc.scalar.activation(rz, s2, Sigmoid)
            u = tmp_pool.tile([128, 128], F32, tag=row_tag + "u")
            nc.vector.tensor_add(u, pghn, bhhn)
            v = tmp_pool.tile([128, 128], F32, tag=row_tag + "v")
            nc.vector.tensor_mul(v, rz[:, 0:128], u)
            w0 = tmp_pool.tile([128, 128], F32, tag=row_tag + "w0")
            nc.vector.tensor_add(w0, pgi[:, 256:384], bihn)
            t3 = tmp_pool.tile([128, 128], F32, tag=row_tag + "t3")
            nc.vector.tensor_add(t3, w0, v)
            n = tmp_pool.tile([128, 128], F32, tag=row_tag + "n")
            nc.scalar.activation(n, t3, Tanh)
            ew = nc.gpsimd if de_engine == "pool" else nc.vector
            d = tmp_pool.tile([128, 128], F32, tag=row_tag + "d")
            ew.tensor_sub(d, row_prev, n)
            e = tmp_pool.tile([128, 128], F32, tag=row_tag + "e")
            ew.tensor_mul(e, rz[:, 128:256], d)
            row_new = st_pool.tile([128, 128], F32, tag=row_tag)
            ew.tensor_add(row_new, n, e)
            rowT = st_pool.tile([128, 128], F32, tag=row_tag + "T")
            nc.vector.transpose(rowT, row_new)
            return row_new, rowT

        import concourse.bass as bass_mod

        def seq_step_ap(t_):
            # dims [c:4, b:32, j:128] matching SBUF dst [p=(c,b), f=j]
            base = seq[:, t_, :]
            return bass_mod.AP(
                tensor=base.tensor,
                offset=base.offset,
                ap=[[128, 4], [n_steps * H, BL], [1, 128]],
            )

        for t_ in range(n_steps):
            xrow = xrow_pool.tile([128, 128], F32, tag="xrow")
            nc.sync.dma_start(out=xrow, in_=seq_step_ap(t_))
            xT = xt_pool.tile([128, 128], F32, tag="xT")
            nc.vector.transpose(xT, xrow)

            pgi = psum_pool.tile([128, 384], F32, tag="gi")
            pghn = psum_pool.tile([128, 128], F32, tag="ghn")
            mm_group(pgi, xT, wsb["wgi"])
            mm_group_hh(pgi, pghn, gT, wsb["wgh"])
            g_row, gT = cell(
                pgi, pghn, g_row, bsb["brz_g"], bsb["bihn_g"], bsb["bhhn_g"], "g_row"
            )

            pai = psum_pool.tile([128, 384], F32, tag="ai")
            pahn = psum_pool.tile([128, 128], F32, tag="ahn")
            mm_group(pai, gT, wsb["wai"])
            mm_group_hh(pai, pahn, aT, wsb["wah"])
            a_row, aT = cell(
                pai, pahn, a_row, bsb["brz_a"], bsb["bihn_a"], bsb["bhhn_a"], "a_row"
            )

        out_ap = bass_mod.AP(
            tensor=out[:].tensor,
            offset=0,
            ap=[[128, 4], [H, BL], [1, 128]],
        )
        nc.sync.dma_start(out=out_ap, in_=a_row)

    nc.compile()
    return nc


def _get_program(n_steps=T):
    key = ("prog", n_steps)
    if key not in _CACHE:
        _CACHE[key] = _build_program(n_steps)
    return _CACHE[key]


# ---------------------------------------------------------------------------
# Entry point
# ---------------------------------------------------------------------------

def _make_in_maps(inputs):
    seq_emb = np.ascontiguousarray(np.asarray(inputs["seq_emb"], np.float32))
    augru_Wih = np.asarray(inputs["augru_Wih"])
    A1 = augru_Wih[:, :H]
    A2 = augru_Wih[:, H:]
    w_fused = (A1 + A2 @ np.asarray(inputs["v_W"])).astype(np.float32)
    b_ai = (np.asarray(inputs["augru_bih"]) + A2 @ np.asarray(inputs["v_b"])).astype(np.float32)
    b_ah = np.asarray(inputs["augru_bhh"], np.float32)
    gru_bih = np.asarray(inputs["gru_bih"])
    gru_bhh = np.asarray(inputs["gru_bhh"])

    consts = {
        "wgi": _arrange_w(np.asarray(inputs["gru_Wih"])),
        "wgh": _arrange_w(np.asarray(inputs["gru_Whh"])),
        "wai": _arrange_w(w_fused),
        "wah": _arrange_w(np.asarray(inputs["augru_Whh"])),
        "brz_g": _bias_rz(gru_bih + gru_bhh),
        "bihn_g": _bias_n(gru_bih),
        "bhhn_g": _bias_n(gru_bhh),
        "brz_a": _bias_rz(b_ai + b_ah),
        "bihn_a": _bias_n(b_ai),
        "bhhn_a": _bias_n(b_ah),
    }
    return [
        {"seq": seq_emb[c * BL:(c + 1) * BL], **consts} for c in range(N_CORES)
    ]


def _prep_and_run(trace=False, **inputs):
    from concourse.bass_utils import run_bass_kernel_spmd

    in_maps = _make_in_maps(inputs)
    nc = _get_program()
    res = run_bass_kernel_spmd(nc, in_maps, list(range(N_CORES)), trace=trace)
    out = np.concatenate([res.results[c]["out"] for c in range(N_CORES)], axis=0)
    return out.astype(np.float32), res


def kernel(**inputs):
    out, _ = _prep_and_run(**inputs)
    return out


def kernel_traced(**inputs):
    """Like kernel() but profiles the run; returns (output, BassKernelResults)."""
    return _prep_and_run(**inputs, trace=True)


if __name__ == "__main__":
    rng = np.random.default_rng(0)
    ins = {
        "seq_emb": rng.standard_normal((B, T, H), dtype=np.float32),
        "target_emb": rng.standard_normal((B, H), dtype=np.float32),
        "gru_Wih": rng.standard_normal((3 * H, H), dtype=np.float32) * 0.04,
        "gru_Whh": rng.standard_normal((3 * H, H), dtype=np.float32) * 0.04,
        "gru_bih": rng.standard_normal(3 * H).astype(np.float32) * 0.04,
        "gru_bhh": rng.standard_normal(3 * H).astype(np.float32) * 0.04,
        "q_W": rng.standard_normal((H, H), dtype=np.float32) * 0.04,
        "q_b": rng.standard_normal(H).astype(np.float32) * 0.04,
        "k_W": rng.standard_normal((H, H), dtype=np.float32) * 0.04,
        "k_b": rng.standard_normal(H).astype(np.float32) * 0.04,
        "v_W": rng.standard_normal((H, H), dtype=np.float32) * 0.04,
        "v_b": rng.standard_normal(H).astype(np.float32) * 0.04,
        "augru_Wih": rng.standard_normal((3 * H, 2 * H), dtype=np.float32) * 0.04,
        "augru_Whh": rng.standard_normal((3 * H, H), dtype=np.float32) * 0.04,
        "augru_bih": rng.standard_normal(3 * H).astype(np.float32) * 0.04,
        "augru_bhh": rng.standard_normal(3 * H).astype(np.float32) * 0.04,
    }
    o = kernel(**ins)
    print("kernel output", o.shape, o.dtype, float(np.abs(o).max()))

